# revision 2
# baseline (speedup 1.0000x reference)
"""Trainium2 Bass kernel for nn_Always (sliding-window smoothed-min).

The reference "scan" is a sliding-window reduction:
    out[b, t, d] = -(1/5) * log( sum_{k=0..15} exp(-5 * x[b, t-k, d]) )
with x[b, j, d] := x[b, 0, d] for j < 0 (the h0 padding).

Strategy (pure data parallel over 8 cores; 2 batches x 2 tensors per core):
  - All device I/O is bf16: inputs are cast f32->bf16 on the host (halves
    HBM read traffic), outputs are produced in bf16 on device and cast back
    f32 on the host. Tolerance is 2e-2 l2-rel; bf16 keeps us ~1e-3.
  - layout: time tiles of 256 timesteps: t = 256*J + 2*p + i with p the SBUF
    partition and (i, d) in the free dim.
  - ScalarE: E = exp(-5x) (bf16 in -> bf16 out)
  - TensorE: banded-matrix matmuls (bf16 weights x bf16 rhs -> f32 PSUM)
    compute the 16-wide window sum S. With 2 rows/partition the band splits
    into 4 (out-parity, in-parity) weight pairs per class: W_in[oi][ii]
    (within-tile), W_halo[oi][ii] (previous tile), W_first[oi] (t=0 pad).
  - ScalarE: ln(S) from PSUM -> bf16; VectorE: * -1/5 (bf16)
  Scheduling structure: all 8 input DMAs are emitted first on the SP
  sequencer, activations run in pinned groups of 4 exps / 4 lns, and all
  output DMAs are emitted last.
"""

import numpy as np

B, T, D = 16, 8192, 64
N_CORES = 8
B_PER_CORE = B // N_CORES  # 2
SCALE = 5.0
WIN = 16
P = 128                     # SBUF partitions
ROWS = 2                    # timesteps per partition per tile
TILE_T = P * ROWS           # 256 timesteps per tile
TILE_COLS = ROWS * D        # 128 free columns per tile
CHUNK_TILES = 16            # tiles per chunk
CHUNK_COLS = CHUNK_TILES * TILE_COLS       # 2048
CHUNKS_PER_SEQ = T // (TILE_T * CHUNK_TILES)  # 2
HALF = CHUNK_TILES // 2     # 8 tiles per psum bank
N_SEQS = 2 * B_PER_CORE     # 4 sequences per core
N_CHUNKS = N_SEQS * CHUNKS_PER_SEQ  # 8
ACT_GROUP = 4               # chunks per exp/ln activation group


def _weight_mats():
    """Returns the 10 banded matrices, concatenated [128, 1280] in bf16:
    order: W_in[0][0], W_in[0][1], W_in[1][0], W_in[1][1],
           W_halo[0][0], ..., W_halo[1][1], W_first[0], W_first[1].
    Layout convention: lhsT[p_in, p_out]; matmul computes lhsT.T @ rhs."""
    import ml_dtypes

    p = np.arange(P)
    mats = []
    for cls in ("in", "halo"):
        for oi in (0, 1):
            for ii in (0, 1):
                t_out = 2 * p[None, :] + oi
                t_in = 2 * p[:, None] + ii
                dd = t_out - t_in + (TILE_T if cls == "halo" else 0)
                lo = 1 if cls == "halo" else 0
                mats.append(((dd >= lo) & (dd <= WIN - 1)).astype(np.float32))
    for oi in (0, 1):
        wf = np.zeros((P, P), np.float32)
        wf[0, :] = np.maximum(WIN - 1 - (2 * p + oi), 0)
        mats.append(wf)
    return np.concatenate(mats, axis=1).astype(ml_dtypes.bfloat16)


def _build_bass(mode="grouped"):
    from contextlib import ExitStack

    import concourse.bacc as bacc
    import concourse.tile as tile
    from concourse import mybir
    from concourse.tile import add_dep_helper

    f32 = mybir.dt.float32
    bf16 = mybir.dt.bfloat16
    AF = mybir.ActivationFunctionType

    nc = bacc.Bacc(trn_type="TRN2")
    lo = nc.dram_tensor("lower", [B_PER_CORE, T, D], bf16, kind="ExternalInput")
    up = nc.dram_tensor("upper", [B_PER_CORE, T, D], bf16, kind="ExternalInput")
    out_lo = nc.dram_tensor("out_lower", [B_PER_CORE, T, D], bf16, kind="ExternalOutput")
    out_up = nc.dram_tensor("out_upper", [B_PER_CORE, T, D], bf16, kind="ExternalOutput")

    w_all_d = nc.inline_tensor(_weight_mats(), name="w_all_c")

    def view3(ap):
        return ap.rearrange("p (J i d) -> p J i d", i=ROWS, d=D)

    with tile.TileContext(nc) as tc, ExitStack() as ctx:
        consts = ctx.enter_context(tc.tile_pool(name="consts", bufs=1))
        x_pool = ctx.enter_context(tc.tile_pool(name="x", bufs=7))
        e_pool = ctx.enter_context(tc.tile_pool(name="e", bufs=N_CHUNKS))
        o_pool = ctx.enter_context(tc.tile_pool(name="o", bufs=4))
        ps_pool = ctx.enter_context(tc.tile_pool(name="ps", bufs=2, space="PSUM"))

        w_all = consts.tile([P, 10 * P], bf16)

        def w(idx):
            return w_all[:, idx * P : (idx + 1) * P]

        W_IN = lambda oi, ii: w(oi * 2 + ii)          # noqa: E731
        W_HALO = lambda oi, ii: w(4 + oi * 2 + ii)    # noqa: E731
        W_FIRST = lambda oi: w(8 + oi)                # noqa: E731

        # chunk list: (dram_x_view, dram_y_view, chunk_idx_within_seq)
        chunks = []
        for src, dst in ((lo, out_lo), (up, out_up)):
            for b in range(B_PER_CORE):
                xv = src[b].rearrange("(J p i) d -> p J i d", p=P, i=ROWS)
                yv = dst[b].rearrange("(J p i) d -> p J i d", p=P, i=ROWS)
                for c in range(CHUNKS_PER_SEQ):
                    chunks.append((xv, yv, c))

        def emit_in(q):
            xv, _yv, c = chunks[q]
            J0 = c * CHUNK_TILES
            xt = x_pool.tile([P, CHUNK_COLS], bf16)
            if q == 0:
                # split the first load so the first exp can start sooner
                for h in (0, 1):
                    nc.sync.dma_start(
                        view3(xt[:])[:, h * HALF : (h + 1) * HALF, :, :],
                        xv[:, J0 + h * HALF : J0 + (h + 1) * HALF, :, :],
                    )
            else:
                nc.sync.dma_start(view3(xt[:]), xv[:, J0 : J0 + CHUNK_TILES, :, :])
            xts.append(xt)

        exp_insts = {}
        ln_insts = {}

        def emit_exp(q):
            # cols [0, TILE_COLS) hold the previous tile (halo); the chunk's
            # 16 tiles follow. Halo matmuls then read one tile-shifted views
            # with no extra split at the chunk boundary.
            _xv, _yv, c = chunks[q]
            et = e_pool.tile([P, TILE_COLS + CHUNK_COLS], bf16)
            if c > 0:
                nc.vector.tensor_copy(
                    et[:, 0:TILE_COLS], ets[q - 1][:, CHUNK_COLS:]
                )
            if q == 0:
                HC = HALF * TILE_COLS
                nc.scalar.activation(
                    et[:, TILE_COLS : TILE_COLS + HC],
                    xts[q][:, 0:HC], AF.Exp, scale=-SCALE,
                )
                exp_insts[q] = nc.scalar.activation(
                    et[:, TILE_COLS + HC :], xts[q][:, HC:], AF.Exp, scale=-SCALE
                ).ins
            else:
                exp_insts[q] = nc.scalar.activation(
                    et[:, TILE_COLS:], xts[q][:], AF.Exp, scale=-SCALE
                ).ins
            ets.append(et)

        xts = []
        ets = []
        if mode == "grouped":
            nc.sync.dma_start(w_all[:], w_all_d[:])
            for q in range(N_CHUNKS):
                emit_in(q)

        # ---- phase B: compute, activation-grouped
        pss = [None] * N_CHUNKS
        ots = [None] * N_CHUNKS

        def emit_mms(q):
            _xv, _yv, c = chunks[q]
            et3 = view3(ets[q][:, TILE_COLS:])
            hl3 = view3(ets[q][:, 0:CHUNK_COLS])  # tile-shifted (halo) view
            ps = ps_pool.tile([P, CHUNK_COLS], f32)
            pss[q] = ps

            mms = []

            def out_ap(oi, j_lo, j_hi):
                return ps[:, oi * 1024 + j_lo * D : oi * 1024 + j_hi * D]

            for oi in (0, 1):
                for ii in (0, 1):
                    lh = W_IN(oi, ii)
                    for h in (0, 1):
                        mms.append((
                            (oi, h), lh,
                            et3[:, h * HALF : (h + 1) * HALF, ii, :],
                            out_ap(oi, h * HALF, (h + 1) * HALF),
                        ))
            for oi in (0, 1):
                for ii in (0, 1):
                    lh = W_HALO(oi, ii)
                    if c > 0:
                        mms.append((
                            (oi, 0), lh,
                            hl3[:, 0:HALF, ii, :],
                            out_ap(oi, 0, HALF),
                        ))
                    else:
                        # first chunk: no halo tile; tiles 0..6 feed outputs
                        # 1..7 (tile 0's pad handled by W_first below)
                        mms.append((
                            (oi, 0), lh,
                            et3[:, 0 : HALF - 1, ii, :],
                            out_ap(oi, 1, HALF),
                        ))
                    mms.append((
                        (oi, 1), lh,
                        hl3[:, HALF:CHUNK_TILES, ii, :],
                        out_ap(oi, HALF, CHUNK_TILES),
                    ))
            if c == 0:
                for oi in (0, 1):
                    mms.append((
                        (oi, 0), W_FIRST(oi),
                        et3[:, 0:1, 0, :],
                        out_ap(oi, 0, 1),
                    ))

            first_seen, last_idx = set(), {}
            for k, (bank, *_rest) in enumerate(mms):
                last_idx[bank] = k
            for k, (bank, lh, rhs, outp) in enumerate(mms):
                st = bank not in first_seen
                first_seen.add(bank)
                nc.tensor.matmul(outp, lh, rhs, start=st, stop=(last_idx[bank] == k))

        def emit_ln(q):
            ot = o_pool.tile([P, CHUNK_COLS], bf16)
            ots[q] = ot
            # ps iterates (oi, J, d); ot memory layout is (J, i, d)
            ps4 = pss[q][:].rearrange("p (oi J d) -> p oi J d", oi=2, d=D)
            ot4 = ot[:].rearrange("p (J i d) -> p i J d", i=ROWS, d=D)
            if q == N_CHUNKS - 1:
                for h in (0, 1):
                    sl = slice(h * HALF, (h + 1) * HALF)
                    ln_insts[q] = nc.scalar.activation(
                        ot4[:, :, sl, :], ps4[:, :, sl, :], AF.Ln
                    ).ins
                    nc.vector.tensor_scalar_mul(
                        ot[:, h * HALF * TILE_COLS : (h + 1) * HALF * TILE_COLS],
                        ot[:, h * HALF * TILE_COLS : (h + 1) * HALF * TILE_COLS],
                        -1.0 / SCALE,
                    )
            else:
                ln_insts[q] = nc.scalar.activation(ot4, pss[q][:], AF.Ln).ins
                nc.vector.tensor_scalar_mul(ot[:], ot[:], -1.0 / SCALE)

        out_insts = {}

        def emit_out(q, engine=None):
            _xv, yv, c = chunks[q]
            J0 = c * CHUNK_TILES
            eng = engine if engine is not None else nc.sync
            if q == N_CHUNKS - 1:
                for h in (0, 1):
                    out_insts[q] = eng.dma_start(
                        yv[:, J0 + h * HALF : J0 + (h + 1) * HALF, :, :],
                        view3(ots[q][:])[:, h * HALF : (h + 1) * HALF, :, :],
                    ).ins
            else:
                out_insts[q] = eng.dma_start(
                    yv[:, J0 : J0 + CHUNK_TILES, :, :], view3(ots[q][:])
                ).ins

        if mode == "grouped":
            for g in range(0, N_CHUNKS, ACT_GROUP):
                grp = list(range(g, min(g + ACT_GROUP, N_CHUNKS)))
                for q in grp:
                    emit_exp(q)
                for q in grp:
                    emit_mms(q)
                for q in grp:
                    emit_ln(q)
                # pin ACT order within/between groups so Exp and Ln table
                # sets switch once per phase (4 loads total), not per chunk
                for q in grp:
                    for q2 in grp:
                        add_dep_helper(
                            ln_insts[q], exp_insts[q2], sync=False,
                            reason="act table grouping",
                        )
                if g > 0:
                    for q in grp:
                        for q2 in range(g - ACT_GROUP, g):
                            add_dep_helper(
                                exp_insts[q], ln_insts[q2], sync=False,
                                reason="act table grouping",
                            )
            for q in range(N_CHUNKS):
                emit_out(q)
        elif mode == "perchunk":
            nc.sync.dma_start(w_all[:], w_all_d[:])
            for q in range(N_CHUNKS):
                emit_in(q)
                emit_exp(q)
                emit_mms(q)
                emit_ln(q)
                emit_out(q)
        elif mode == "insfirst":
            nc.sync.dma_start(w_all[:], w_all_d[:])
            for q in range(N_CHUNKS):
                emit_in(q)
            for q in range(N_CHUNKS):
                emit_exp(q)
                emit_mms(q)
                emit_ln(q)
            for q in range(N_CHUNKS):
                emit_out(q)
        else:
            raise ValueError(mode)
    nc.compile()
    return nc


def _run(lower_trace, upper_trace, trace=False, mode="grouped", **spmd_kwargs):
    import ml_dtypes

    from concourse.bass_utils import run_bass_kernel_spmd

    bf = ml_dtypes.bfloat16
    lower_trace = np.asarray(lower_trace, dtype=np.float32).astype(bf)
    upper_trace = np.asarray(upper_trace, dtype=np.float32).astype(bf)
    assert lower_trace.shape == (B, T, D) and upper_trace.shape == (B, T, D)

    nc = _build_bass(mode=mode)
    in_maps = [
        {
            "lower": np.ascontiguousarray(lower_trace[i * B_PER_CORE : (i + 1) * B_PER_CORE]),
            "upper": np.ascontiguousarray(upper_trace[i * B_PER_CORE : (i + 1) * B_PER_CORE]),
        }
        for i in range(N_CORES)
    ]
    res = run_bass_kernel_spmd(
        nc, in_maps, core_ids=list(range(N_CORES)), trace=trace, **spmd_kwargs
    )
    out_lower = np.concatenate(
        [r["out_lower"].astype(np.float32) for r in res.results], axis=0
    )
    out_upper = np.concatenate(
        [r["out_upper"].astype(np.float32) for r in res.results], axis=0
    )
    return (out_lower, out_upper), res


def kernel(lower_trace, upper_trace):
    (out_lower, out_upper), _ = _run(lower_trace, upper_trace, trace=False)
    return out_lower, out_upper


# revision 3
# speedup vs baseline: 1.4876x; 1.4876x over previous
"""Trainium2 Bass kernel for nn_Always (sliding-window smoothed-min).

The reference "scan" is a sliding-window reduction:
    out[b, t, d] = -(1/5) * log( sum_{k=0..15} exp(-5 * x[b, t-k, d]) )
with x[b, j, d] := x[b, 0, d] for j < 0 (the h0 padding).

Strategy (pure data parallel over 8 cores; 2 batches x 2 tensors per core):
  - All device I/O is bf16 and HOST-PERMUTED into the compute layout, so
    every DMA descriptor is a 2-4 KB contiguous run (vs 256 B in the
    naive [t, d] layout). The host does x[b].reshape(64, 128, 64)
    .transpose(1, 0, 2): partition p holds timesteps t = 128*J + p,
    free axis is (J, d). The inverse permute runs on the host after.
  - VectorE (DVE): E = exp(-5x) via a Schraudolph bit-trick entirely in
    16-bit: i16 = round(A*x + B) reinterpreted as bf16 gives 2^(A'x+B')
    with ~+-9% worst-case rel err (incl. bf16 input rounding), which the
    smoothed-min output absorbs to ~2e-3 l2 rel err (tolerance is 2e-2).
    This moves exp off the Scalar engine, which is the throughput floor.
  - TensorE: banded matmuls (bf16) compute the 16-wide window sum S.
    R=1 layout means ONE in-band matrix W_in (po-pi in [0,15]) and one
    halo matrix W_halo (reads the previous 128-step tile via a shifted
    view of the same buffer -- no copies), W_first handles t<16 padding.
  - ScalarE: only ln(S) from PSUM -> bf16 (one ACT table set, no swaps).
  - VectorE: * -1/5 (bf16).
"""

import numpy as np

B, T, D = 16, 8192, 64
N_CORES = 8
B_PER_CORE = B // N_CORES  # 2
SCALE = 5.0
WIN = 16
P = 128                    # SBUF partitions; tile = 128 timesteps (R=1)
SEQ_TILES = T // P         # 64 tiles per sequence
SEQ_COLS = SEQ_TILES * D   # 4096 free columns per sequence
N_SEQS = 2 * B_PER_CORE    # 4 sequences per core (2 tensors x 2 batches)
PC_TILES = 32              # tiles per PSUM chunk
PC_COLS = PC_TILES * D     # 2048 cols = 8 KB f32 = 4 PSUM banks
N_PC = SEQ_TILES // PC_TILES  # 2 PSUM chunks per sequence
QT = 8                     # tiles per PSUM bank (matmul granularity)
QCOLS = QT * D             # 512 cols = 2 KB f32 = 1 bank

# Schraudolph exp constants: i16 = A*x + B, bits(i16) read as bf16
# approximates 2^(-5*log2(e)*x) = exp(-5x). c=0.0579 centers the
# piecewise-linear mantissa error.
EXP_A = float(-5.0 * np.log2(np.e) * 128.0)
EXP_B = float(128.0 * (127.0 - 0.0579))


def _weight_mats():
    """[128, 384] bf16: W_in | W_halo | W_first.
    Layout convention: lhsT[p_in, p_out]; matmul computes lhsT.T @ rhs."""
    import ml_dtypes

    p = np.arange(P)
    dd = p[None, :] - p[:, None]  # p_out - p_in
    w_in = ((dd >= 0) & (dd <= WIN - 1)).astype(np.float32)
    # halo: input from previous tile, dd_eff = dd + 128 in [1, 15]
    w_halo = ((dd + P >= 1) & (dd + P <= WIN - 1)).astype(np.float32)
    # first tile of a sequence: taps at t<0 all read x[0] (partition 0)
    w_first = np.zeros((P, P), np.float32)
    w_first[0, :] = np.maximum(WIN - 1 - p, 0)
    return np.concatenate([w_in, w_halo, w_first], axis=1).astype(
        ml_dtypes.bfloat16
    )


def _build_bass(mode="grouped"):
    from contextlib import ExitStack

    import concourse.bacc as bacc
    import concourse.tile as tile
    from concourse import mybir

    f32 = mybir.dt.float32
    bf16 = mybir.dt.bfloat16
    i16 = mybir.dt.int16
    AF = mybir.ActivationFunctionType
    ALU = mybir.AluOpType

    nc = bacc.Bacc(trn_type="TRN2")
    xin = nc.dram_tensor("xin", [N_SEQS, P, SEQ_COLS], bf16, kind="ExternalInput")
    yout = nc.dram_tensor("yout", [N_SEQS, P, SEQ_COLS], bf16, kind="ExternalOutput")
    w_all_d = nc.inline_tensor(_weight_mats(), name="w_all_c")

    with tile.TileContext(nc) as tc, ExitStack() as ctx:
        consts = ctx.enter_context(tc.tile_pool(name="consts", bufs=1))
        x_pool = ctx.enter_context(tc.tile_pool(name="x", bufs=N_SEQS))
        e_pool = ctx.enter_context(tc.tile_pool(name="e", bufs=N_SEQS))
        o_pool = ctx.enter_context(tc.tile_pool(name="o", bufs=4))
        ps_pool = ctx.enter_context(tc.tile_pool(name="ps", bufs=2, space="PSUM"))

        w_all = consts.tile([P, 3 * P], bf16)
        W_IN = w_all[:, 0:P]
        W_HALO = w_all[:, P : 2 * P]
        W_FIRST = w_all[:, 2 * P : 3 * P]

        nc.sync.dma_start(w_all[:], w_all_d[:])

        # ---- input DMAs, all emitted first on the SP sequencer
        xts = []
        for s in range(N_SEQS):
            xt = x_pool.tile([P, SEQ_COLS], bf16)
            nparts = 4 if s == 0 else 2  # fine-grain the first so exp starts early
            step = SEQ_COLS // nparts
            for h in range(nparts):
                nc.sync.dma_start(
                    xt[:, h * step : (h + 1) * step],
                    xin[s][:, h * step : (h + 1) * step],
                )
            xts.append(xt)

        # ---- DVE exp (all emitted before the ln-muls in DVE program order)
        ets = []
        for s in range(N_SEQS):
            et = e_pool.tile([P, SEQ_COLS], bf16)
            for h in range(2):
                sl = slice(h * PC_COLS, (h + 1) * PC_COLS)
                nc.vector.tensor_scalar(
                    et[:, sl].bitcast(i16),
                    xts[s][:, sl],
                    EXP_A,
                    EXP_B,
                    op0=ALU.mult,
                    op1=ALU.add,
                )
            ets.append(et)

        # ---- matmul window-sums + ln + scale
        ots = {}
        for s in range(N_SEQS):
            et3 = ets[s][:].rearrange("p (J d) -> p J d", d=D)
            for pc in range(N_PC):
                ps = ps_pool.tile([P, PC_COLS], f32)
                for m in range(4):
                    J0 = pc * PC_TILES + m * QT
                    outp = ps[:, m * QCOLS : (m + 1) * QCOLS]
                    nc.tensor.matmul(
                        outp, W_IN, et3[:, J0 : J0 + QT, :], start=True, stop=False
                    )
                    if J0 == 0:
                        # no previous tile: tiles 0..6 feed out-tiles 1..7;
                        # the t<16 padding taps come from W_first
                        nc.tensor.matmul(
                            ps[:, D:QCOLS], W_HALO, et3[:, 0 : QT - 1, :],
                            start=False, stop=False,
                        )
                        nc.tensor.matmul(
                            ps[:, 0:D], W_FIRST, et3[:, 0:1, :],
                            start=False, stop=True,
                        )
                    else:
                        nc.tensor.matmul(
                            outp, W_HALO, et3[:, J0 - 1 : J0 + QT - 1, :],
                            start=False, stop=True,
                        )
                ot = o_pool.tile([P, PC_COLS], bf16)
                nc.scalar.activation(ot[:], ps[:], AF.Ln)
                nc.vector.tensor_scalar_mul(ot[:], ot[:], -1.0 / SCALE)
                ots[(s, pc)] = ot

        # ---- output DMAs, all emitted last
        for s in range(N_SEQS):
            for pc in range(N_PC):
                nc.sync.dma_start(
                    yout[s][:, pc * PC_COLS : (pc + 1) * PC_COLS], ots[(s, pc)][:]
                )
    nc.compile()
    return nc


def _permute_in(x):
    """[B, T, D] f32 -> [B, P, SEQ_COLS] bf16 with t = 128*J + p."""
    import ml_dtypes

    return np.ascontiguousarray(
        np.asarray(x, dtype=np.float32)
        .reshape(B, SEQ_TILES, P, D)
        .transpose(0, 2, 1, 3)
        .reshape(B, P, SEQ_COLS)
    ).astype(ml_dtypes.bfloat16)


def _permute_out(y):
    """[P, SEQ_COLS] bf16 -> [T, D] f32 (inverse of _permute_in per seq)."""
    return (
        np.asarray(y)
        .astype(np.float32)
        .reshape(P, SEQ_TILES, D)
        .transpose(1, 0, 2)
        .reshape(T, D)
    )


def _run(lower_trace, upper_trace, trace=False, mode="grouped", **spmd_kwargs):
    from concourse.bass_utils import run_bass_kernel_spmd

    lp = _permute_in(lower_trace)
    up = _permute_in(upper_trace)

    nc = _build_bass(mode=mode)
    in_maps = []
    for i in range(N_CORES):
        b0, b1 = 2 * i, 2 * i + 1
        in_maps.append(
            {"xin": np.ascontiguousarray(np.stack([lp[b0], lp[b1], up[b0], up[b1]]))}
        )
    res = run_bass_kernel_spmd(
        nc, in_maps, core_ids=list(range(N_CORES)), trace=trace, **spmd_kwargs
    )
    out_lower = np.empty((B, T, D), np.float32)
    out_upper = np.empty((B, T, D), np.float32)
    for i in range(N_CORES):
        y = res.results[i]["yout"]
        out_lower[2 * i] = _permute_out(y[0])
        out_lower[2 * i + 1] = _permute_out(y[1])
        out_upper[2 * i] = _permute_out(y[2])
        out_upper[2 * i + 1] = _permute_out(y[3])
    return (out_lower, out_upper), res


def kernel(lower_trace, upper_trace):
    (out_lower, out_upper), _ = _run(lower_trace, upper_trace, trace=False)
    return out_lower, out_upper
